# revision 40
# baseline (speedup 1.0000x reference)
"""HGCN layer kernel for Trainium2, 8 NeuronCores, row-sharded SPMD.

Reference computation (N=6144, D=512):
    type_sum_a = adj_a @ x ; type_sum_b = adj_b @ x
    attn_a = sigmoid(cat[ts_a, x] @ Wa.T + ba) ; attn_b likewise
    h = x @ W_sa ; s_l = h @ a_sa[:512] ; s_r = h @ a_sa[512:]
    scores[i,j] = s_l[i] + s_r[j]
    e = adj_a * exp(-leaky_relu(scores, 0.01)) ; attn = e / (rowsum(e)+1e-5)
    x_a = attn @ h ; x_b = adj_b @ (x @ W_gcnb) + b_gcnb
    out = sigmoid(attn_a * x_a + attn_b * x_b)

Kernel strategy v2 (per core, NL=768 local rows):
  - Reassociate the big matmuls so they contract against x directly:
        x_a = (attn @ x) @ W_sa      x_b = (adj_b @ x) @ W_gcnb
    This removes the replicated all-N computation of h = x@W_sa and
    x@W_gcnb (the v1 Phase A), which cost ~100us of PE per core.
  - Per-node scalar stats (s_l, s_r, gate vectors) come from a skinny
    12-column matmul streamed over x^T in fp16 (lhsT = x^T tiles).
  - Main N-contraction matmuls produce taT/tbT in [feature, local-node]
    layout: lhsT = natural-x tiles (bf16), rhs = e^T / adj^T tiles.
    Small post-matmuls (lhsT = taT/tbT chunks, rhs = W rows) restore
    the natural [node, feature] layout for the combine.
  - e computed in transposed layout [j(part), i(free)]; adjacency passed
    pre-transposed, per-core-permuted (local rows first).
  - rowsum/ga/gb gate rows via zero-padded 2-row lhsT side-passes.
  - e-pipeline alternates engines by j parity: even j uses scalar Prelu
    (bias-fused add), odd j uses vector add+max, to balance ACT vs DVE.
"""

import numpy as np
from contextlib import ExitStack

import concourse.bass as bass
import concourse.bacc as bacc
import concourse.mybir as mybir
import concourse.tile as tile

F32 = mybir.dt.float32
F16 = mybir.dt.float16
BF16 = mybir.dt.bfloat16
AF = mybir.ActivationFunctionType
ALU = mybir.AluOpType

N_CORES = 8
NS = 10  # stats columns per node tile (see make_cols_matrix)


def build_program(n, d, nl, ba, bb, dt_x=F16, dt_bc=BF16):
    """Build the SPMD Bass program. Returns nc.

    n: total nodes, d: feature dim, nl: local rows per core.
    ba/bb: python-float gate biases (baked in).
    """
    JT = n // 128   # j tiles (contraction/node axis)
    IT = nl // 128  # local row tiles
    KT = d // 128   # feature k tiles
    FT = KT         # feature tiles of taT/tbT partition dim
    # nl split into psum-bank-sized chunks
    LO = 512        # first chunk width
    HI = nl - LO    # second chunk width (256)

    nc = bacc.Bacc("TRN2", target_bir_lowering=False, debug=False,
                   num_devices=N_CORES)

    Q = 4           # j tiles per DMA transfer (fat lines, few dma_starts)
    JQ = JT // Q

    xt_dram = nc.dram_tensor("xt", [JQ, 128, Q * d], dt_x, kind="ExternalInput")
    xn_dram = nc.dram_tensor("xn", [JQ, 128, Q * d], dt_bc, kind="ExternalInput")
    cols_dram = nc.dram_tensor("cols", [128, KT * NS], dt_x, kind="ExternalInput")
    # wmat: KT chunks of [W_sa | W_gcnb] rows, then KT cols of Wb1 (for gb)
    wmat_dram = nc.dram_tensor("wmat", [128, KT * 2 * d + KT], dt_bc,
                               kind="ExternalInput")
    adjat_dram = nc.dram_tensor("adjat", [JQ, 128, Q * nl], dt_bc, kind="ExternalInput")
    adjbt_dram = nc.dram_tensor("adjbt", [JQ, 128, Q * nl], dt_bc, kind="ExternalInput")
    bbias_dram = nc.dram_tensor("bbias", [128, d], F32, kind="ExternalInput")
    ident_dram = nc.dram_tensor("ident", [128, 128], F32, kind="ExternalInput")
    out_dram = nc.dram_tensor("out", [nl, d], F32, kind="ExternalOutput")

    def mm(out, lhsT, rhs, start, stop, skip=False):
        nc.tensor.matmul(out, lhsT, rhs, start=start, stop=stop,
                         skip_group_check=skip)

    with tile.TileContext(nc) as tc, ExitStack() as ctx:
        const = ctx.enter_context(tc.tile_pool(name="const", bufs=1))
        # C-phase adjacency pool opened up-front (own SBUF region) so its
        # first DMAs prefetch during phase B with no WAR dependence.
        adjc = ctx.enter_context(tc.tile_pool(name="adjC", bufs=3))

        xn_sb = const.tile([128, JT * d], dt_bc, tag="xn")
        cols_sb = const.tile([128, KT * NS], dt_x, tag="cols")
        wmat_sb = const.tile([128, KT * 2 * d + KT], dt_bc, tag="wmat")
        stats_sb = const.tile([128, JT * NS], F32, tag="stats")
        stats_r = const.tile([128, JT * NS], dt_bc, tag="stats_r")
        slb_sb = const.tile([128, nl], F32, tag="slb")
        taT_sb = const.tile([128, FT * nl], dt_bc, tag="taT")
        tbT_sb = const.tile([128, FT * nl], dt_bc, tag="tbT")
        xbf_sb = const.tile([128, IT * d], F32, tag="xbf")
        bbias_sb = const.tile([128, d], F32, tag="bbias")
        ident_sb = const.tile([128, 128], F32, tag="ident")
        onespad = const.tile([128, 2], dt_bc, tag="onespad")
        ones4 = const.tile([128, 4], dt_bc, tag="ones4")
        onespad_f = const.tile([128, 4], F32, tag="onespadf")
        ones_row = const.tile([1, 128], F32, tag="ones_r")
        neg1 = const.tile([128, 1], F32, tag="neg1")
        ba_sb = const.tile([128, 1], F32, tag="ba")
        bb_sb = const.tile([128, 1], F32, tag="bb")
        sl_row = const.tile([1, nl], F32, tag="sl_row")
        # rgx rows: 0=rs[0:LO] 1=ga[0:LO] 2=rs[LO:nl] 3=ga[LO:nl]
        rgx_sb = const.tile([4, LO], F32, tag="rgx")
        # g_sb cols [4c:4c+4] = (rs_c, ga_c, rs_{4+c}, ga_{4+c}) transposes
        g_sb = const.tile([128, 4 * 4], F32, tag="g")
        gate_sb = const.tile([128, 4 * IT], F32, tag="gate")
        # gate_sb cols: [0:IT]=recip(rowsum), [IT:2IT]=sig_a, [2IT:3IT]=sig_b,
        # [3IT:4IT]=scratch

        nc.sync.dma_start(out=cols_sb[:], in_=cols_dram[:])
        nc.sync.dma_start(out=ident_sb[:], in_=ident_dram[:])
        # wmat/bbias (needed from the B->C boundary) are DMA'd mid-loop so
        # they don't compete with the critical first xt/adj transfers.
        nc.vector.memset(onespad_f[:], 0.0)
        nc.vector.memset(onespad_f[:, 0:1], 1.0)
        nc.vector.tensor_copy(onespad[:], onespad_f[:, 0:2])
        # ones4 = (0,0,1,0) selects psum row 2 for the hi rowsum
        nc.vector.memset(onespad_f[:], 0.0)
        nc.vector.memset(onespad_f[:, 2:3], 1.0)
        nc.vector.tensor_copy(ones4[:], onespad_f[:])
        nc.vector.memset(ones_row[:], 1.0)
        nc.vector.memset(neg1[:], -1.0)
        nc.vector.memset(ba_sb[:], float(ba))
        nc.vector.memset(bb_sb[:], float(bb))
        # preload the Sigmoid activation table off the critical path
        nc.scalar.activation(neg1[:], neg1[:], AF.Sigmoid)
        nc.vector.memset(neg1[:], -1.0)

        # ---- Phase S+B fused: stats (x @ cols) and tbT = (adj_b@x)^T ----
        # Per j step: stats for node-tile j (lhsT = x^T tile fp16), then
        # the adj_b contraction (lhsT = natural-x chunks, rhs = adjbT).
        # gb = vb^T adj_bT accumulated in SBUF via a transient psum bank
        # with zero-padded (vb,0)/(0,vb) weight pairs.
        with tc.tile_pool(name="xt_pool", bufs=2) as xtp, \
             tc.tile_pool(name="adj_pool", bufs=3) as adjp, \
             tc.tile_pool(name="psS", bufs=1, space="PSUM") as psS, \
             tc.tile_pool(name="psB", bufs=1, space="PSUM") as psB:
            ptb_lo = [psB.tile([128, LO], F32, tag=f"ptbl{f}", name=f"ptbl{f}")
                      for f in range(FT)]
            ptb_hi = [psB.tile([128, 2 * HI], F32, tag=f"ptbh{g}",
                               name=f"ptbh{g}") for g in range(FT // 2)]
            for jq in range(JQ):
                xt_t = xtp.tile([128, Q * d], dt_x, tag="xt")
                if jq == 0:
                    # split the first transfer so stats j=0 starts sooner
                    nc.sync.dma_start(out=xt_t[:, 0:d], in_=xt_dram[0, :, 0:d])
                    nc.sync.dma_start(out=xt_t[:, d:Q * d],
                                      in_=xt_dram[0, :, d:Q * d])
                else:
                    nc.sync.dma_start(out=xt_t[:], in_=xt_dram[jq])
                nc.sync.dma_start(out=xn_sb[:, jq * Q * d:(jq + 1) * Q * d],
                                  in_=xn_dram[jq])
                aq = adjp.tile([128, Q * nl], dt_bc, tag="adj")
                nc.sync.dma_start(out=aq[:], in_=adjbt_dram[jq])
                if jq == 2:
                    nc.sync.dma_start(out=wmat_sb[:], in_=wmat_dram[:])
                    nc.sync.dma_start(out=bbias_sb[:], in_=bbias_dram[:])
                for s in range(Q):
                    j = jq * Q + s
                    # stats for node tile j
                    ps = psS.tile([128, NS], F32, tag="ps")
                    for k in range(KT):
                        mm(ps[:], xt_t[:, s * d + k * 128:s * d + (k + 1) * 128],
                           cols_sb[:, k * NS:(k + 1) * NS],
                           k == 0, k == KT - 1)
                    nc.vector.tensor_copy(stats_sb[:, j * NS:(j + 1) * NS], ps[:])
                    nc.vector.tensor_copy(stats_r[:, j * NS:(j + 1) * NS], ps[:])
                    # adj_b contraction
                    at = aq[:, s * nl:(s + 1) * nl]
                    st, sp = (j == 0), (j == JT - 1)
                    for f in range(FT):
                        lhsT = xn_sb[:, j * d + f * 128:j * d + (f + 1) * 128]
                        mm(ptb_lo[f][:], lhsT, at[:, 0:LO], st, sp)
                        g, h = divmod(f, 2)
                        # shared bank: only the h==0 group's first mm zeroes it
                        mm(ptb_hi[g][:, h * HI:(h + 1) * HI], lhsT,
                           at[:, LO:nl], st and h == 0, sp, skip=True)
                    # (gb needs no side-pass: gb = Wb1^T @ tbT in phase D)
            for f in range(FT):
                # split psum drain across scalar and vector engines
                if f % 2 == 0:
                    nc.scalar.copy(tbT_sb[:, f * nl:f * nl + LO], ptb_lo[f][:])
                else:
                    nc.vector.tensor_copy(tbT_sb[:, f * nl:f * nl + LO],
                                          ptb_lo[f][:])
            for g in range(FT // 2):
                for h in range(2):
                    f = 2 * g + h
                    if f % 2 == 0:
                        nc.scalar.copy(tbT_sb[:, f * nl + LO:(f + 1) * nl],
                                       ptb_hi[g][:, h * HI:(h + 1) * HI])
                    else:
                        nc.vector.tensor_copy(tbT_sb[:, f * nl + LO:(f + 1) * nl],
                                              ptb_hi[g][:, h * HI:(h + 1) * HI])

        # ---- Phase A2: build SL broadcast [128, nl] from local s_l ----
        with tc.tile_pool(name="psA2", bufs=1, space="PSUM") as psA2:
            pr0 = psA2.tile([1, LO], F32, tag="pr0")
            pr1 = psA2.tile([1, HI], F32, tag="pr1")
            for t in range(IT):
                off = t * 128
                tgt = (pr0[0:1, off:off + 128] if off < LO
                       else pr1[0:1, off - LO:off - LO + 128])
                # one group per bank: first write zeroes, last write stops
                nc.tensor.matmul(tgt, stats_sb[:, t * NS:t * NS + 1],
                                 ident_sb[:], start=(off % LO == 0),
                                 stop=(off in (LO - 128, nl - 128)),
                                 skip_group_check=True)
            nc.vector.tensor_copy(sl_row[0:1, 0:LO], pr0[0:1, :])
            nc.vector.tensor_copy(sl_row[0:1, LO:nl], pr1[0:1, :])
            pb0 = psA2.tile([128, LO], F32, tag="pb0")
            pb1 = psA2.tile([128, HI], F32, tag="pb1")
            nc.tensor.matmul(pb0[:], ones_row[:], sl_row[0:1, 0:LO],
                             start=True, stop=True)
            nc.tensor.matmul(pb1[:], ones_row[:], sl_row[0:1, LO:nl],
                             start=True, stop=True)
            nc.vector.tensor_copy(slb_sb[:, 0:LO], pb0[:])
            nc.vector.tensor_copy(slb_sb[:, LO:nl], pb1[:])

        # ---- Phase C: e = adj_a * exp(-lrelu(s)); taT = x^T e ----
        with tc.tile_pool(name="ewC", bufs=4) as ewp, \
             tc.tile_pool(name="psC", bufs=1, space="PSUM") as psC:
            adjp = adjc
            pta_lo = [psC.tile([128, LO], F32, tag=f"ptal{f}", name=f"ptal{f}")
                      for f in range(FT)]
            pta_hi = [psC.tile([128, 2 * HI], F32, tag=f"ptah{g}",
                               name=f"ptah{g}") for g in range(FT // 2)]
            # rs/ga rows for both nl-chunks packed in one bank:
            # rows 0,1 = rs,ga of [0:LO] (2-wide lhsT); rows 2,3 = rs,ga of
            # [LO:nl] (4-wide zero-padded lhsT). The bank is zeroed by DVE
            # and every matmul accumulates (no start flags): correct whether
            # or not a start would clear beyond its own out region.
            prg = psC.tile([4, LO], F32, tag="prg")
            nc.vector.memset(prg[0:4, :], 0.0)
            # x_b post-matmuls fill the B->C boundary PE bubble: they only
            # need tbT (ready) and run while the first e tiles are produced.
            for i in range(IT):
                pxb = psC.tile([128, d], F32, tag="pxb")
                for f in range(FT):
                    mm(pxb[:], tbT_sb[:, f * nl + i * 128:f * nl + (i + 1) * 128],
                       wmat_sb[:, f * 2 * d + d:(f + 1) * 2 * d],
                       f == 0, f == FT - 1)
                # fused bias add while draining psum
                nc.vector.tensor_tensor(xbf_sb[:, i * d:(i + 1) * d], pxb[:],
                                        bbias_sb[:], op=ALU.add)
            for jq in range(JQ):
                aq = adjp.tile([128, Q * nl], dt_bc, tag="adj")
                nc.sync.dma_start(out=aq[:], in_=adjat_dram[jq])
                for s in range(Q):
                    j = jq * Q + s
                    at = aq[:, s * nl:(s + 1) * nl]
                    s_r = stats_sb[:, j * NS + 1:j * NS + 2]
                    m_t = ewp.tile([128, nl], F32, tag="m")
                    # split m = lrelu(s_l + s_r) by column halves so ACT and
                    # DVE carry even loads every j (no parity jitter):
                    # vector: cols [0:SPL), scalar (bias-fused Prelu): rest
                    SPL = 384
                    nc.vector.tensor_scalar_add(m_t[:, 0:SPL],
                                                slb_sb[:, 0:SPL], s_r)
                    nc.vector.scalar_tensor_tensor(m_t[:, 0:SPL],
                                                   m_t[:, 0:SPL], 0.01,
                                                   m_t[:, 0:SPL],
                                                   op0=ALU.mult, op1=ALU.max)
                    nc.scalar.activation(m_t[:, SPL:nl], slb_sb[:, SPL:nl],
                                         AF.Prelu, bias=s_r, alpha=0.01)
                    # w = exp(-m), in place
                    nc.scalar.activation(m_t[:], m_t[:], AF.Exp, scale=neg1[:])
                    e_t = ewp.tile([128, nl], dt_bc, tag="e")
                    nc.vector.tensor_tensor(e_t[:], m_t[:], at[:], op=ALU.mult)
                    st, sp = (j == 0), (j == JT - 1)
                    for f in range(FT):
                        lhsT = xn_sb[:, j * d + f * 128:j * d + (f + 1) * 128]
                        mm(pta_lo[f][:], lhsT, e_t[:, 0:LO], st, sp)
                        g, h = divmod(f, 2)
                        # shared bank: only h==0 group's first mm zeroes it
                        mm(pta_hi[g][:, h * HI:(h + 1) * HI], lhsT,
                           e_t[:, LO:nl], st and h == 0, sp, skip=True)
                    # rowsum/ga side-passes, all four rows in one bank
                    zva = stats_r[:, j * NS + 2:j * NS + 4]    # (0, va)
                    va4 = stats_r[:, j * NS + 6:j * NS + 10]   # (0,0,0,va)
                    mm(prg[0:2, :], onespad[:], e_t[:, 0:LO], False, False,
                       skip=True)
                    mm(prg[0:4, 0:HI], ones4[:], e_t[:, LO:nl], False, False,
                       skip=True)
                    mm(prg[0:2, :], zva, at[:, 0:LO], False, False, skip=True)
                    mm(prg[0:4, 0:HI], va4, at[:, LO:nl], False, sp, skip=True)
            for f in range(FT):
                if f % 2 == 0:
                    nc.scalar.copy(taT_sb[:, f * nl:f * nl + LO], pta_lo[f][:])
                else:
                    nc.vector.tensor_copy(taT_sb[:, f * nl:f * nl + LO],
                                          pta_lo[f][:])
            for g in range(FT // 2):
                for h in range(2):
                    f = 2 * g + h
                    if f % 2 == 0:
                        nc.scalar.copy(taT_sb[:, f * nl + LO:(f + 1) * nl],
                                       pta_hi[g][:, h * HI:(h + 1) * HI])
                    else:
                        nc.vector.tensor_copy(taT_sb[:, f * nl + LO:(f + 1) * nl],
                                              pta_hi[g][:, h * HI:(h + 1) * HI])
            nc.vector.tensor_copy(rgx_sb[0:4, :], prg[0:4, :])

        # ---- Phase D: gate transposes, gb/x_a post-matmuls, combine ----
        with tc.tile_pool(name="psD", bufs=1, space="PSUM") as psD, \
             tc.tile_pool(name="psD2", bufs=2, space="PSUM") as psD2, \
             tc.tile_pool(name="outD", bufs=2) as outp:
            pT = psD.tile([128, 4 * 4], F32, tag="pT")
            for c in range(4):
                # transpose all four rgx rows: pT[4c:4c+4] =
                # (rs_c, ga_c, rs_{4+c}, ga_{4+c})  (c>=2: hi cols unused)
                nc.tensor.matmul(pT[:, 4 * c:4 * c + 4],
                                 rgx_sb[0:4, c * 128:(c + 1) * 128],
                                 ident_sb[0:4, 0:4], start=(c == 0),
                                 stop=(c == 3), skip_group_check=True)
            nc.vector.tensor_copy(g_sb[:], pT[:])

            def rs_col(i):
                return 4 * i if i < 4 else 4 * (i - 4) + 2

            def ga_col(i):
                return 4 * i + 1 if i < 4 else 4 * (i - 4) + 3
            for i in range(IT):
                # gb = Wb1^T @ tbT chunk (natural layout, no transpose)
                pgb = psD2.tile([128, 1], F32, tag="pgb")
                for f in range(FT):
                    mm(pgb[:], tbT_sb[:, f * nl + i * 128:f * nl + (i + 1) * 128],
                       wmat_sb[:, KT * 2 * d + f:KT * 2 * d + f + 1],
                       f == 0, f == FT - 1)
                # recip(rowsum + 1e-5)
                nc.vector.tensor_scalar_add(gate_sb[:, 3 * IT + i:3 * IT + i + 1],
                                            g_sb[:, rs_col(i):rs_col(i) + 1],
                                            1e-5)
                nc.vector.reciprocal(gate_sb[:, i:i + 1],
                                     gate_sb[:, 3 * IT + i:3 * IT + i + 1])
                # sig_a = sigmoid(ga + wa2x + ba)
                nc.vector.tensor_tensor(gate_sb[:, 3 * IT + i:3 * IT + i + 1],
                                        g_sb[:, ga_col(i):ga_col(i) + 1],
                                        stats_sb[:, i * NS + 4:i * NS + 5],
                                        op=ALU.add)
                nc.scalar.activation(gate_sb[:, IT + i:IT + i + 1],
                                     gate_sb[:, 3 * IT + i:3 * IT + i + 1],
                                     AF.Sigmoid, bias=ba_sb[:])
                # sig_b = sigmoid(gb + wb2x + bb)
                nc.vector.tensor_tensor(gate_sb[:, 3 * IT + i:3 * IT + i + 1],
                                        pgb[:],
                                        stats_sb[:, i * NS + 5:i * NS + 6],
                                        op=ALU.add)
                nc.scalar.activation(gate_sb[:, 2 * IT + i:2 * IT + i + 1],
                                     gate_sb[:, 3 * IT + i:3 * IT + i + 1],
                                     AF.Sigmoid, bias=bb_sb[:])
            for i in range(IT):
                pxa = psD2.tile([128, d], F32, tag="pxa")
                for f in range(FT):
                    lha = taT_sb[:, f * nl + i * 128:f * nl + (i + 1) * 128]
                    mm(pxa[:], lha, wmat_sb[:, f * 2 * d:f * 2 * d + d],
                       f == 0, f == FT - 1)
                u_t = outp.tile([128, d], F32, tag="u")
                # u = sig_a * (x_a_raw * recip)
                nc.vector.tensor_scalar(u_t[:], pxa[:],
                                        gate_sb[:, i:i + 1],
                                        gate_sb[:, IT + i:IT + i + 1],
                                        op0=ALU.mult, op1=ALU.mult)
                t_t = outp.tile([128, d], F32, tag="t")
                # y = sigmoid((x_b_raw + b_gcnb) * sig_b + u)
                nc.vector.scalar_tensor_tensor(t_t[:],
                                               xbf_sb[:, i * d:(i + 1) * d],
                                               gate_sb[:, 2 * IT + i:2 * IT + i + 1],
                                               u_t[:], op0=ALU.mult, op1=ALU.add)
                y_t = outp.tile([128, d], F32, tag="y")
                nc.scalar.activation(y_t[:], t_t[:], AF.Sigmoid)
                nc.sync.dma_start(out=out_dram[i * 128:(i + 1) * 128, :],
                                  in_=y_t[:])

    nc.compile()
    return nc


def make_cols_matrix(W_sa, a_sa, Wa, Wb, d):
    """Stats weight columns [d, NS].

    0=s_l, 1=s_r, 2=0, 3=va(Wa1), 4=wa2x(Wa2), 5=wb2x(Wb2), 6..8=0, 9=va.
    Slices used as lhsT: [2:4]=(0,va) for the lo ga pass, [6:10]=(0,0,0,va)
    for the hi ga pass (4-row psum bank pack).
    """
    cols = np.zeros((d, NS), dtype=np.float32)
    cols[:, 0] = W_sa @ a_sa[0, :d]
    cols[:, 1] = W_sa @ a_sa[0, d:]
    cols[:, 3] = Wa[0, :d]
    cols[:, 4] = Wa[0, d:]
    cols[:, 5] = Wb[0, d:]
    cols[:, 9] = Wa[0, :d]
    return cols


def make_core_inputs(x, adj_a, adj_b, cols, wmat, wb1, b_gcnb, n, d, nl, core,
                     np_x=np.float16, np_bc=None):
    if np_bc is None:
        import ml_dtypes
        np_bc = ml_dtypes.bfloat16
    JT, KT = n // 128, d // 128
    Q = 4
    JQ = JT // Q
    rows = np.arange(core * nl, (core + 1) * nl)
    perm = np.concatenate([rows, np.arange(0, core * nl),
                           np.arange((core + 1) * nl, n)])
    xp = x[perm]
    # xt[j, p, k*128+c] = xp[j*128+c, k*128+p]  (x^T tiles), then Q tiles
    # packed per DMA row: [jq, p, s*d + k*128 + c]
    xt = (xp.reshape(JT, 128, KT, 128).transpose(0, 3, 2, 1)
          .reshape(JQ, Q, 128, d).transpose(0, 2, 1, 3).reshape(JQ, 128, Q * d))
    xn = (xp.reshape(JQ, Q, 128, d).transpose(0, 2, 1, 3)
          .reshape(JQ, 128, Q * d))
    adjat = (adj_a[rows][:, perm].T.reshape(JQ, Q, 128, nl)
             .transpose(0, 2, 1, 3).reshape(JQ, 128, Q * nl))
    adjbt = (adj_b[rows][:, perm].T.reshape(JQ, Q, 128, nl)
             .transpose(0, 2, 1, 3).reshape(JQ, 128, Q * nl))
    # cols/wmat packed k-major along free dim: [p, k*W + c]; wmat gains
    # KT trailing cols of Wb1 chunks (for gb = Wb1^T @ tbT)
    colsr = cols.reshape(KT, 128, NS).transpose(1, 0, 2).reshape(128, KT * NS)
    wmatr = np.concatenate([
        wmat.transpose(1, 0, 2).reshape(128, KT * 2 * d),
        wb1.reshape(KT, 128).T], axis=1)
    return {
        "xt": np.ascontiguousarray(xt).astype(np_x),
        "xn": np.ascontiguousarray(xn).astype(np_bc),
        "cols": np.ascontiguousarray(colsr).astype(np_x),
        "wmat": np.ascontiguousarray(wmatr).astype(np_bc),
        "adjat": np.ascontiguousarray(adjat).astype(np_bc),
        "adjbt": np.ascontiguousarray(adjbt).astype(np_bc),
        "bbias": np.ascontiguousarray(
            np.broadcast_to(b_gcnb, (128, d))).astype(np.float32),
        "ident": np.eye(128, dtype=np.float32),
    }


_CACHE = {}


def _install_ntff_hook():
    """Dev-only: register the axon NTFF profile hook so trace=True works.

    The agent image's antenv package lacks axon_hooks; synthesize it and
    wire trn_boot's ctypes-based hook to /opt/axon/libaxon_pjrt.so.
    """
    import sys
    import types
    try:
        from antenv import axon_hooks  # noqa: F401
        return
    except ImportError:
        pass
    import antenv
    mod = types.ModuleType("antenv.axon_hooks")
    _h = [None]
    mod.get_axon_ntff_profile_hook = lambda: _h[0]
    mod.set_axon_ntff_profile_hook = lambda hook: _h.__setitem__(0, hook)
    sys.modules["antenv.axon_hooks"] = mod
    antenv.axon_hooks = mod
    from trn_agent_boot.trn_boot import _ntff_profile_via_ctypes
    mod.set_axon_ntff_profile_hook(
        _ntff_profile_via_ctypes("/opt/axon/libaxon_pjrt.so"))


def kernel(x, adj_a, adj_b, W_sa, a_sa, W_gcnb, b_gcnb, Wa, ba, Wb, bb,
           _trace=False, _trace_kwargs=None):
    from concourse.bass_utils import run_bass_kernel_spmd
    if _trace:
        _install_ntff_hook()

    n, d = x.shape
    nl = n // N_CORES
    cols = make_cols_matrix(W_sa, a_sa, Wa, Wb, d)
    wmat = np.ascontiguousarray(
        np.concatenate([W_sa, W_gcnb], axis=1)
        .reshape(d // 128, 128, 2 * d)).astype(np.float32)

    key = (n, d, nl, float(ba[0]), float(bb[0]))
    if key not in _CACHE:
        _CACHE[key] = build_program(n, d, nl, float(ba[0]), float(bb[0]))
    nc = _CACHE[key]

    in_maps = [make_core_inputs(x, adj_a, adj_b, cols, wmat, Wb[0, :d],
                                b_gcnb, n, d, nl, c)
               for c in range(N_CORES)]
    res = run_bass_kernel_spmd(nc, in_maps, list(range(N_CORES)),
                               trace=_trace, **(_trace_kwargs or {}))
    out = np.empty((n, d), dtype=np.float32)
    for c in range(N_CORES):
        out[c * nl:(c + 1) * nl] = res.results[c]["out"]
    if _trace:
        kernel._last_results = res
    return out


# revision 42
# speedup vs baseline: 1.0271x; 1.0271x over previous
"""HGCN layer kernel for Trainium2, 8 NeuronCores, row-sharded SPMD.

Reference computation (N=6144, D=512):
    type_sum_a = adj_a @ x ; type_sum_b = adj_b @ x
    attn_a = sigmoid(cat[ts_a, x] @ Wa.T + ba) ; attn_b likewise
    h = x @ W_sa ; s_l = h @ a_sa[:512] ; s_r = h @ a_sa[512:]
    scores[i,j] = s_l[i] + s_r[j]
    e = adj_a * exp(-leaky_relu(scores, 0.01)) ; attn = e / (rowsum(e)+1e-5)
    x_a = attn @ h ; x_b = adj_b @ (x @ W_gcnb) + b_gcnb
    out = sigmoid(attn_a * x_a + attn_b * x_b)

Kernel strategy v2 (per core, NL=768 local rows):
  - Reassociate the big matmuls so they contract against x directly:
        x_a = (attn @ x) @ W_sa      x_b = (adj_b @ x) @ W_gcnb
    This removes the replicated all-N computation of h = x@W_sa and
    x@W_gcnb (the v1 Phase A), which cost ~100us of PE per core.
  - Per-node scalar stats (s_l, s_r, gate vectors) come from a skinny
    12-column matmul streamed over x^T in fp16 (lhsT = x^T tiles).
  - Main N-contraction matmuls produce taT/tbT in [feature, local-node]
    layout: lhsT = natural-x tiles (bf16), rhs = e^T / adj^T tiles.
    Small post-matmuls (lhsT = taT/tbT chunks, rhs = W rows) restore
    the natural [node, feature] layout for the combine.
  - e computed in transposed layout [j(part), i(free)]; adjacency passed
    pre-transposed, per-core-permuted (local rows first).
  - rowsum/ga/gb gate rows via zero-padded 2-row lhsT side-passes.
  - e-pipeline alternates engines by j parity: even j uses scalar Prelu
    (bias-fused add), odd j uses vector add+max, to balance ACT vs DVE.
"""

import numpy as np
from contextlib import ExitStack

import concourse.bass as bass
import concourse.bacc as bacc
import concourse.mybir as mybir
import concourse.tile as tile

F32 = mybir.dt.float32
F16 = mybir.dt.float16
BF16 = mybir.dt.bfloat16
AF = mybir.ActivationFunctionType
ALU = mybir.AluOpType

N_CORES = 8
NS = 10  # stats columns per node tile (see make_cols_matrix)


def build_program(n, d, nl, ba, bb, dt_x=F16, dt_bc=BF16):
    """Build the SPMD Bass program. Returns nc.

    n: total nodes, d: feature dim, nl: local rows per core.
    ba/bb: python-float gate biases (baked in).
    """
    JT = n // 128   # j tiles (contraction/node axis)
    IT = nl // 128  # local row tiles
    KT = d // 128   # feature k tiles
    FT = KT         # feature tiles of taT/tbT partition dim
    # nl split into psum-bank-sized chunks
    LO = 512        # first chunk width
    HI = nl - LO    # second chunk width (256)

    nc = bacc.Bacc("TRN2", target_bir_lowering=False, debug=False,
                   num_devices=N_CORES)

    Q = 4           # j tiles per DMA transfer (fat lines, few dma_starts)
    JQ = JT // Q

    xt_dram = nc.dram_tensor("xt", [JQ, 128, Q * d], dt_x, kind="ExternalInput")
    xn_dram = nc.dram_tensor("xn", [JQ, 128, Q * d], dt_bc, kind="ExternalInput")
    cols_dram = nc.dram_tensor("cols", [128, KT * NS], dt_x, kind="ExternalInput")
    # wmat: KT chunks of [W_sa | W_gcnb] rows, then KT cols of Wb1 (for gb)
    wmat_dram = nc.dram_tensor("wmat", [128, KT * 2 * d + KT], dt_bc,
                               kind="ExternalInput")
    adjat_dram = nc.dram_tensor("adjat", [JQ, 128, Q * nl], dt_bc, kind="ExternalInput")
    adjbt_dram = nc.dram_tensor("adjbt", [JQ, 128, Q * nl], dt_bc, kind="ExternalInput")
    bbias_dram = nc.dram_tensor("bbias", [128, d], F32, kind="ExternalInput")
    ident_dram = nc.dram_tensor("ident", [128, 128], F32, kind="ExternalInput")
    out_dram = nc.dram_tensor("out", [nl, d], F32, kind="ExternalOutput")

    def mm(out, lhsT, rhs, start, stop, skip=False):
        nc.tensor.matmul(out, lhsT, rhs, start=start, stop=stop,
                         skip_group_check=skip)

    with tile.TileContext(nc) as tc, ExitStack() as ctx:
        const = ctx.enter_context(tc.tile_pool(name="const", bufs=1))
        # C-phase adjacency pool opened up-front (own SBUF region) so its
        # first DMAs prefetch during phase B with no WAR dependence.
        adjc = ctx.enter_context(tc.tile_pool(name="adjC", bufs=3))

        xn_sb = const.tile([128, JT * d], dt_bc, tag="xn")
        cols_sb = const.tile([128, KT * NS], dt_x, tag="cols")
        wmat_sb = const.tile([128, KT * 2 * d + KT], dt_bc, tag="wmat")
        stats_sb = const.tile([128, JT * NS], F32, tag="stats")
        stats_r = const.tile([128, JT * NS], dt_bc, tag="stats_r")
        slb_sb = const.tile([128, nl], F32, tag="slb")
        taT_sb = const.tile([128, FT * nl], dt_bc, tag="taT")
        tbT_sb = const.tile([128, FT * nl], dt_bc, tag="tbT")
        xbf_sb = const.tile([128, IT * d], F32, tag="xbf")
        bbias_sb = const.tile([128, d], F32, tag="bbias")
        ident_sb = const.tile([128, 128], F32, tag="ident")
        onespad = const.tile([128, 2], dt_bc, tag="onespad")
        ones4 = const.tile([128, 4], dt_bc, tag="ones4")
        onespad_f = const.tile([128, 4], F32, tag="onespadf")
        ones_row = const.tile([1, 128], F32, tag="ones_r")
        neg1 = const.tile([128, 1], F32, tag="neg1")
        ba_sb = const.tile([128, 1], F32, tag="ba")
        bb_sb = const.tile([128, 1], F32, tag="bb")
        sl_row = const.tile([1, nl], F32, tag="sl_row")
        # rgx rows: 0=rs[0:LO] 1=ga[0:LO] 2=rs[LO:nl] 3=ga[LO:nl]
        rgx_sb = const.tile([4, LO], F32, tag="rgx")
        # g_sb cols [4c:4c+4] = (rs_c, ga_c, rs_{4+c}, ga_{4+c}) transposes
        g_sb = const.tile([128, 4 * 4], F32, tag="g")
        gate_sb = const.tile([128, 4 * IT], F32, tag="gate")
        # gate_sb cols: [0:IT]=recip(rowsum), [IT:2IT]=sig_a, [2IT:3IT]=sig_b,
        # [3IT:4IT]=scratch

        nc.sync.dma_start(out=cols_sb[:], in_=cols_dram[:])
        nc.sync.dma_start(out=ident_sb[:], in_=ident_dram[:])
        # wmat/bbias (needed from the B->C boundary) are DMA'd mid-loop so
        # they don't compete with the critical first xt/adj transfers.
        nc.vector.memset(onespad_f[:], 0.0)
        nc.vector.memset(onespad_f[:, 0:1], 1.0)
        nc.vector.tensor_copy(onespad[:], onespad_f[:, 0:2])
        # ones4 = (0,0,1,0) selects psum row 2 for the hi rowsum
        nc.vector.memset(onespad_f[:], 0.0)
        nc.vector.memset(onespad_f[:, 2:3], 1.0)
        nc.vector.tensor_copy(ones4[:], onespad_f[:])
        nc.vector.memset(ones_row[:], 1.0)
        nc.vector.memset(neg1[:], -1.0)
        nc.vector.memset(ba_sb[:], float(ba))
        nc.vector.memset(bb_sb[:], float(bb))
        # preload the Sigmoid activation table off the critical path
        nc.scalar.activation(neg1[:], neg1[:], AF.Sigmoid)
        nc.vector.memset(neg1[:], -1.0)

        # ---- Phase S+B fused: stats (x @ cols) and tbT = (adj_b@x)^T ----
        # Per j step: stats for node-tile j (lhsT = x^T tile fp16), then
        # the adj_b contraction (lhsT = natural-x chunks, rhs = adjbT).
        # gb = vb^T adj_bT accumulated in SBUF via a transient psum bank
        # with zero-padded (vb,0)/(0,vb) weight pairs.
        with tc.tile_pool(name="xt_pool", bufs=2) as xtp, \
             tc.tile_pool(name="adj_pool", bufs=3) as adjp, \
             tc.tile_pool(name="psS", bufs=1, space="PSUM") as psS, \
             tc.tile_pool(name="psB", bufs=1, space="PSUM") as psB:
            ptb_lo = [psB.tile([128, LO], F32, tag=f"ptbl{f}", name=f"ptbl{f}")
                      for f in range(FT)]
            ptb_hi = [psB.tile([128, 2 * HI], F32, tag=f"ptbh{g}",
                               name=f"ptbh{g}") for g in range(FT // 2)]
            for jq in range(JQ):
                xt_t = xtp.tile([128, Q * d], dt_x, tag="xt")
                if jq == 0:
                    # split the first transfer so stats j=0 starts sooner
                    nc.sync.dma_start(out=xt_t[:, 0:d], in_=xt_dram[0, :, 0:d])
                    nc.sync.dma_start(out=xt_t[:, d:Q * d],
                                      in_=xt_dram[0, :, d:Q * d])
                else:
                    nc.sync.dma_start(out=xt_t[:], in_=xt_dram[jq])
                nc.sync.dma_start(out=xn_sb[:, jq * Q * d:(jq + 1) * Q * d],
                                  in_=xn_dram[jq])
                aq = adjp.tile([128, Q * nl], dt_bc, tag="adj")
                nc.sync.dma_start(out=aq[:], in_=adjbt_dram[jq])
                if jq == 6:
                    W2 = (KT * 2 * d + KT) // 2
                    nc.sync.dma_start(out=wmat_sb[:, 0:W2],
                                      in_=wmat_dram[:, 0:W2])
                    nc.sync.dma_start(out=bbias_sb[:], in_=bbias_dram[:])
                elif jq == 8:
                    W2 = (KT * 2 * d + KT) // 2
                    nc.sync.dma_start(out=wmat_sb[:, W2:],
                                      in_=wmat_dram[:, W2:])
                for s in range(Q):
                    j = jq * Q + s
                    # stats for node tile j
                    ps = psS.tile([128, NS], F32, tag="ps")
                    for k in range(KT):
                        mm(ps[:], xt_t[:, s * d + k * 128:s * d + (k + 1) * 128],
                           cols_sb[:, k * NS:(k + 1) * NS],
                           k == 0, k == KT - 1)
                    nc.vector.tensor_copy(stats_sb[:, j * NS:(j + 1) * NS], ps[:])
                    nc.vector.tensor_copy(stats_r[:, j * NS:(j + 1) * NS], ps[:])
                    # adj_b contraction
                    at = aq[:, s * nl:(s + 1) * nl]
                    st, sp = (j == 0), (j == JT - 1)
                    for f in range(FT):
                        lhsT = xn_sb[:, j * d + f * 128:j * d + (f + 1) * 128]
                        mm(ptb_lo[f][:], lhsT, at[:, 0:LO], st, sp)
                        g, h = divmod(f, 2)
                        # shared bank: only the h==0 group's first mm zeroes it
                        mm(ptb_hi[g][:, h * HI:(h + 1) * HI], lhsT,
                           at[:, LO:nl], st and h == 0, sp, skip=True)
                    # (gb needs no side-pass: gb = Wb1^T @ tbT in phase D)
            for f in range(FT):
                # split psum drain across scalar and vector engines
                if f % 2 == 0:
                    nc.scalar.copy(tbT_sb[:, f * nl:f * nl + LO], ptb_lo[f][:])
                else:
                    nc.vector.tensor_copy(tbT_sb[:, f * nl:f * nl + LO],
                                          ptb_lo[f][:])
            for g in range(FT // 2):
                for h in range(2):
                    f = 2 * g + h
                    if f % 2 == 0:
                        nc.scalar.copy(tbT_sb[:, f * nl + LO:(f + 1) * nl],
                                       ptb_hi[g][:, h * HI:(h + 1) * HI])
                    else:
                        nc.vector.tensor_copy(tbT_sb[:, f * nl + LO:(f + 1) * nl],
                                              ptb_hi[g][:, h * HI:(h + 1) * HI])

        # ---- Phase A2: build SL broadcast [128, nl] from local s_l ----
        with tc.tile_pool(name="psA2", bufs=1, space="PSUM") as psA2:
            pr0 = psA2.tile([1, LO], F32, tag="pr0")
            pr1 = psA2.tile([1, HI], F32, tag="pr1")
            for t in range(IT):
                off = t * 128
                tgt = (pr0[0:1, off:off + 128] if off < LO
                       else pr1[0:1, off - LO:off - LO + 128])
                # one group per bank: first write zeroes, last write stops
                nc.tensor.matmul(tgt, stats_sb[:, t * NS:t * NS + 1],
                                 ident_sb[:], start=(off % LO == 0),
                                 stop=(off in (LO - 128, nl - 128)),
                                 skip_group_check=True)
            nc.vector.tensor_copy(sl_row[0:1, 0:LO], pr0[0:1, :])
            nc.vector.tensor_copy(sl_row[0:1, LO:nl], pr1[0:1, :])
            pb0 = psA2.tile([128, LO], F32, tag="pb0")
            pb1 = psA2.tile([128, HI], F32, tag="pb1")
            nc.tensor.matmul(pb0[:], ones_row[:], sl_row[0:1, 0:LO],
                             start=True, stop=True)
            nc.tensor.matmul(pb1[:], ones_row[:], sl_row[0:1, LO:nl],
                             start=True, stop=True)
            nc.vector.tensor_copy(slb_sb[:, 0:LO], pb0[:])
            nc.vector.tensor_copy(slb_sb[:, LO:nl], pb1[:])

        # ---- Phase C: e = adj_a * exp(-lrelu(s)); taT = x^T e ----
        with tc.tile_pool(name="ewC", bufs=4) as ewp, \
             tc.tile_pool(name="psC", bufs=1, space="PSUM") as psC:
            adjp = adjc
            pta_lo = [psC.tile([128, LO], F32, tag=f"ptal{f}", name=f"ptal{f}")
                      for f in range(FT)]
            pta_hi = [psC.tile([128, 2 * HI], F32, tag=f"ptah{g}",
                               name=f"ptah{g}") for g in range(FT // 2)]
            # rs/ga rows for both nl-chunks packed in one bank:
            # rows 0,1 = rs,ga of [0:LO] (2-wide lhsT); rows 2,3 = rs,ga of
            # [LO:nl] (4-wide zero-padded lhsT). The bank is zeroed by DVE
            # and every matmul accumulates (no start flags): correct whether
            # or not a start would clear beyond its own out region.
            prg = psC.tile([4, LO], F32, tag="prg")
            nc.vector.memset(prg[0:4, :], 0.0)
            # x_b post-matmuls fill the B->C boundary PE bubble: they only
            # need tbT (ready) and run while the first e tiles are produced.
            for i in range(IT):
                pxb = psC.tile([128, d], F32, tag="pxb")
                for f in range(FT):
                    mm(pxb[:], tbT_sb[:, f * nl + i * 128:f * nl + (i + 1) * 128],
                       wmat_sb[:, f * 2 * d + d:(f + 1) * 2 * d],
                       f == 0, f == FT - 1)
                # fused bias add while draining psum
                nc.vector.tensor_tensor(xbf_sb[:, i * d:(i + 1) * d], pxb[:],
                                        bbias_sb[:], op=ALU.add)
            for jq in range(JQ):
                aq = adjp.tile([128, Q * nl], dt_bc, tag="adj")
                nc.sync.dma_start(out=aq[:], in_=adjat_dram[jq])
                for s in range(Q):
                    j = jq * Q + s
                    at = aq[:, s * nl:(s + 1) * nl]
                    s_r = stats_sb[:, j * NS + 1:j * NS + 2]
                    m_t = ewp.tile([128, nl], F32, tag="m")
                    if j % 2 == 0:
                        # scalar path: prelu with bias-fused add
                        nc.scalar.activation(m_t[:], slb_sb[:], AF.Prelu,
                                             bias=s_r, alpha=0.01)
                    else:
                        # vector path: add + leaky-relu via mult/max
                        nc.vector.tensor_scalar_add(m_t[:], slb_sb[:], s_r)
                        nc.vector.scalar_tensor_tensor(m_t[:], m_t[:], 0.01,
                                                       m_t[:], op0=ALU.mult,
                                                       op1=ALU.max)
                    # w = exp(-m), in place
                    nc.scalar.activation(m_t[:], m_t[:], AF.Exp, scale=neg1[:])
                    e_t = ewp.tile([128, nl], dt_bc, tag="e")
                    nc.vector.tensor_tensor(e_t[:], m_t[:], at[:], op=ALU.mult)
                    st, sp = (j == 0), (j == JT - 1)
                    for f in range(FT):
                        lhsT = xn_sb[:, j * d + f * 128:j * d + (f + 1) * 128]
                        mm(pta_lo[f][:], lhsT, e_t[:, 0:LO], st, sp)
                        g, h = divmod(f, 2)
                        # shared bank: only h==0 group's first mm zeroes it
                        mm(pta_hi[g][:, h * HI:(h + 1) * HI], lhsT,
                           e_t[:, LO:nl], st and h == 0, sp, skip=True)
                    # rowsum/ga side-passes, all four rows in one bank
                    zva = stats_r[:, j * NS + 2:j * NS + 4]    # (0, va)
                    va4 = stats_r[:, j * NS + 6:j * NS + 10]   # (0,0,0,va)
                    mm(prg[0:2, :], onespad[:], e_t[:, 0:LO], False, False,
                       skip=True)
                    mm(prg[0:4, 0:HI], ones4[:], e_t[:, LO:nl], False, False,
                       skip=True)
                    mm(prg[0:2, :], zva, at[:, 0:LO], False, False, skip=True)
                    mm(prg[0:4, 0:HI], va4, at[:, LO:nl], False, sp, skip=True)
            for f in range(FT):
                if f % 2 == 0:
                    nc.scalar.copy(taT_sb[:, f * nl:f * nl + LO], pta_lo[f][:])
                else:
                    nc.vector.tensor_copy(taT_sb[:, f * nl:f * nl + LO],
                                          pta_lo[f][:])
            for g in range(FT // 2):
                for h in range(2):
                    f = 2 * g + h
                    if f % 2 == 0:
                        nc.scalar.copy(taT_sb[:, f * nl + LO:(f + 1) * nl],
                                       pta_hi[g][:, h * HI:(h + 1) * HI])
                    else:
                        nc.vector.tensor_copy(taT_sb[:, f * nl + LO:(f + 1) * nl],
                                              pta_hi[g][:, h * HI:(h + 1) * HI])
            nc.vector.tensor_copy(rgx_sb[0:4, :], prg[0:4, :])

        # ---- Phase D: gate transposes, gb/x_a post-matmuls, combine ----
        with tc.tile_pool(name="psD", bufs=1, space="PSUM") as psD, \
             tc.tile_pool(name="psD2", bufs=2, space="PSUM") as psD2, \
             tc.tile_pool(name="outD", bufs=2) as outp:
            pT = psD.tile([128, 4 * 4], F32, tag="pT")
            for c in range(4):
                # transpose all four rgx rows: pT[4c:4c+4] =
                # (rs_c, ga_c, rs_{4+c}, ga_{4+c})  (c>=2: hi cols unused)
                nc.tensor.matmul(pT[:, 4 * c:4 * c + 4],
                                 rgx_sb[0:4, c * 128:(c + 1) * 128],
                                 ident_sb[0:4, 0:4], start=(c == 0),
                                 stop=(c == 3), skip_group_check=True)
            nc.vector.tensor_copy(g_sb[:], pT[:])

            def rs_col(i):
                return 4 * i if i < 4 else 4 * (i - 4) + 2

            def ga_col(i):
                return 4 * i + 1 if i < 4 else 4 * (i - 4) + 3
            for i in range(IT):
                # gb = Wb1^T @ tbT chunk (natural layout, no transpose)
                pgb = psD2.tile([128, 1], F32, tag="pgb")
                for f in range(FT):
                    mm(pgb[:], tbT_sb[:, f * nl + i * 128:f * nl + (i + 1) * 128],
                       wmat_sb[:, KT * 2 * d + f:KT * 2 * d + f + 1],
                       f == 0, f == FT - 1)
                # recip(rowsum + 1e-5)
                nc.vector.tensor_scalar_add(gate_sb[:, 3 * IT + i:3 * IT + i + 1],
                                            g_sb[:, rs_col(i):rs_col(i) + 1],
                                            1e-5)
                nc.vector.reciprocal(gate_sb[:, i:i + 1],
                                     gate_sb[:, 3 * IT + i:3 * IT + i + 1])
                # sig_a = sigmoid(ga + wa2x + ba)
                nc.vector.tensor_tensor(gate_sb[:, 3 * IT + i:3 * IT + i + 1],
                                        g_sb[:, ga_col(i):ga_col(i) + 1],
                                        stats_sb[:, i * NS + 4:i * NS + 5],
                                        op=ALU.add)
                nc.scalar.activation(gate_sb[:, IT + i:IT + i + 1],
                                     gate_sb[:, 3 * IT + i:3 * IT + i + 1],
                                     AF.Sigmoid, bias=ba_sb[:])
                # sig_b = sigmoid(gb + wb2x + bb)
                nc.vector.tensor_tensor(gate_sb[:, 3 * IT + i:3 * IT + i + 1],
                                        pgb[:],
                                        stats_sb[:, i * NS + 5:i * NS + 6],
                                        op=ALU.add)
                nc.scalar.activation(gate_sb[:, 2 * IT + i:2 * IT + i + 1],
                                     gate_sb[:, 3 * IT + i:3 * IT + i + 1],
                                     AF.Sigmoid, bias=bb_sb[:])
            for i in range(IT):
                pxa = psD2.tile([128, d], F32, tag="pxa")
                for f in range(FT):
                    lha = taT_sb[:, f * nl + i * 128:f * nl + (i + 1) * 128]
                    mm(pxa[:], lha, wmat_sb[:, f * 2 * d:f * 2 * d + d],
                       f == 0, f == FT - 1)
                u_t = outp.tile([128, d], F32, tag="u")
                # u = sig_a * (x_a_raw * recip)
                nc.vector.tensor_scalar(u_t[:], pxa[:],
                                        gate_sb[:, i:i + 1],
                                        gate_sb[:, IT + i:IT + i + 1],
                                        op0=ALU.mult, op1=ALU.mult)
                t_t = outp.tile([128, d], F32, tag="t")
                # y = sigmoid((x_b_raw + b_gcnb) * sig_b + u)
                nc.vector.scalar_tensor_tensor(t_t[:],
                                               xbf_sb[:, i * d:(i + 1) * d],
                                               gate_sb[:, 2 * IT + i:2 * IT + i + 1],
                                               u_t[:], op0=ALU.mult, op1=ALU.add)
                y_t = outp.tile([128, d], F32, tag="y")
                nc.scalar.activation(y_t[:], t_t[:], AF.Sigmoid)
                nc.sync.dma_start(out=out_dram[i * 128:(i + 1) * 128, :],
                                  in_=y_t[:])

    nc.compile()
    return nc


def make_cols_matrix(W_sa, a_sa, Wa, Wb, d):
    """Stats weight columns [d, NS].

    0=s_l, 1=s_r, 2=0, 3=va(Wa1), 4=wa2x(Wa2), 5=wb2x(Wb2), 6..8=0, 9=va.
    Slices used as lhsT: [2:4]=(0,va) for the lo ga pass, [6:10]=(0,0,0,va)
    for the hi ga pass (4-row psum bank pack).
    """
    cols = np.zeros((d, NS), dtype=np.float32)
    cols[:, 0] = W_sa @ a_sa[0, :d]
    cols[:, 1] = W_sa @ a_sa[0, d:]
    cols[:, 3] = Wa[0, :d]
    cols[:, 4] = Wa[0, d:]
    cols[:, 5] = Wb[0, d:]
    cols[:, 9] = Wa[0, :d]
    return cols


def make_core_inputs(x, adj_a, adj_b, cols, wmat, wb1, b_gcnb, n, d, nl, core,
                     np_x=np.float16, np_bc=None):
    if np_bc is None:
        import ml_dtypes
        np_bc = ml_dtypes.bfloat16
    JT, KT = n // 128, d // 128
    Q = 4
    JQ = JT // Q
    rows = np.arange(core * nl, (core + 1) * nl)
    perm = np.concatenate([rows, np.arange(0, core * nl),
                           np.arange((core + 1) * nl, n)])
    xp = x[perm]
    # xt[j, p, k*128+c] = xp[j*128+c, k*128+p]  (x^T tiles), then Q tiles
    # packed per DMA row: [jq, p, s*d + k*128 + c]
    xt = (xp.reshape(JT, 128, KT, 128).transpose(0, 3, 2, 1)
          .reshape(JQ, Q, 128, d).transpose(0, 2, 1, 3).reshape(JQ, 128, Q * d))
    xn = (xp.reshape(JQ, Q, 128, d).transpose(0, 2, 1, 3)
          .reshape(JQ, 128, Q * d))
    adjat = (adj_a[rows][:, perm].T.reshape(JQ, Q, 128, nl)
             .transpose(0, 2, 1, 3).reshape(JQ, 128, Q * nl))
    adjbt = (adj_b[rows][:, perm].T.reshape(JQ, Q, 128, nl)
             .transpose(0, 2, 1, 3).reshape(JQ, 128, Q * nl))
    # cols/wmat packed k-major along free dim: [p, k*W + c]; wmat gains
    # KT trailing cols of Wb1 chunks (for gb = Wb1^T @ tbT)
    colsr = cols.reshape(KT, 128, NS).transpose(1, 0, 2).reshape(128, KT * NS)
    wmatr = np.concatenate([
        wmat.transpose(1, 0, 2).reshape(128, KT * 2 * d),
        wb1.reshape(KT, 128).T], axis=1)
    return {
        "xt": np.ascontiguousarray(xt).astype(np_x),
        "xn": np.ascontiguousarray(xn).astype(np_bc),
        "cols": np.ascontiguousarray(colsr).astype(np_x),
        "wmat": np.ascontiguousarray(wmatr).astype(np_bc),
        "adjat": np.ascontiguousarray(adjat).astype(np_bc),
        "adjbt": np.ascontiguousarray(adjbt).astype(np_bc),
        "bbias": np.ascontiguousarray(
            np.broadcast_to(b_gcnb, (128, d))).astype(np.float32),
        "ident": np.eye(128, dtype=np.float32),
    }


_CACHE = {}


def _install_ntff_hook():
    """Dev-only: register the axon NTFF profile hook so trace=True works.

    The agent image's antenv package lacks axon_hooks; synthesize it and
    wire trn_boot's ctypes-based hook to /opt/axon/libaxon_pjrt.so.
    """
    import sys
    import types
    try:
        from antenv import axon_hooks  # noqa: F401
        return
    except ImportError:
        pass
    import antenv
    mod = types.ModuleType("antenv.axon_hooks")
    _h = [None]
    mod.get_axon_ntff_profile_hook = lambda: _h[0]
    mod.set_axon_ntff_profile_hook = lambda hook: _h.__setitem__(0, hook)
    sys.modules["antenv.axon_hooks"] = mod
    antenv.axon_hooks = mod
    from trn_agent_boot.trn_boot import _ntff_profile_via_ctypes
    mod.set_axon_ntff_profile_hook(
        _ntff_profile_via_ctypes("/opt/axon/libaxon_pjrt.so"))


def kernel(x, adj_a, adj_b, W_sa, a_sa, W_gcnb, b_gcnb, Wa, ba, Wb, bb,
           _trace=False, _trace_kwargs=None):
    from concourse.bass_utils import run_bass_kernel_spmd
    if _trace:
        _install_ntff_hook()

    n, d = x.shape
    nl = n // N_CORES
    cols = make_cols_matrix(W_sa, a_sa, Wa, Wb, d)
    wmat = np.ascontiguousarray(
        np.concatenate([W_sa, W_gcnb], axis=1)
        .reshape(d // 128, 128, 2 * d)).astype(np.float32)

    key = (n, d, nl, float(ba[0]), float(bb[0]))
    if key not in _CACHE:
        _CACHE[key] = build_program(n, d, nl, float(ba[0]), float(bb[0]))
    nc = _CACHE[key]

    in_maps = [make_core_inputs(x, adj_a, adj_b, cols, wmat, Wb[0, :d],
                                b_gcnb, n, d, nl, c)
               for c in range(N_CORES)]
    res = run_bass_kernel_spmd(nc, in_maps, list(range(N_CORES)),
                               trace=_trace, **(_trace_kwargs or {}))
    out = np.empty((n, d), dtype=np.float32)
    for c in range(N_CORES):
        out[c * nl:(c + 1) * nl] = res.results[c]["out"]
    if _trace:
        kernel._last_results = res
    return out
